# revision 9
# baseline (speedup 1.0000x reference)
"""Trainium2 Bass kernel for CustomizeL2Loss.

Reference computation (x, y: (N, C, T, V, M) = (256, 3, 600, 25, 2) f32):
    motion    = x[:, :, 1:] - x[:, :, :-1]
    mean_move = mean(|motion|, axis=(C, T-1, M))            -> (N, V)
    ratio     = V * mean_move / sum_v(mean_move)            -> (N, V)
    loss      = mean((x - y)**2 * ratio[:, None, None, :, None])

Decomposition used here (linearity):
    S[n, v] = sum_{c,t,m} (x - y)^2          A[n, v] = sum_{c,t,m} |motion|
    loss = (1 / (N*C*T*V*M)) * sum_n V * sum_v(A[n,v] * S[n,v]) / sum_v A[n,v]
(the 1/(C*(T-1)*M) mean_move normalization cancels inside ratio)

Device kernel (data-parallel over batch, 8 cores x 32 samples):
  Per-core layout: x viewed as (96 blocks, 120, 250) where block = (n_local, c),
  partition row p = t-group [5p, 5p+5), free = (t_sub 5, v*m 50). All DRAM reads
  are contiguous 1000B runs. Per 12-block tile:
    d  = x - y                    (VectorE)   sq = Square(d) -> bf16 (ScalarE)
    d1 = x[:, :, 50:] - x[:, :, :-50]  (in-row motion pairs, 4*120 per block)
    xs = partition-shift of x[1:, :, 0:50] (SBUF->SBUF DMA)
    d2 = xs - x[:119, :, 200:250]      (cross-row motion pairs, 119 per block)
    a1, a2 = Abs -> bf16          (ScalarE)
  Reduction over (c, t) via TensorE: ones(K,32) stationary, data as moving rhs,
  PSUM accumulated per sample at partition strip 32*(s%4), bank s//4.
  One ScalarE drain of all of PSUM at the end; host folds (t_sub, m) and does
  the tiny (256, 25) ratio/loss math in float64.
"""

from contextlib import ExitStack

import numpy as np

import concourse.bass as bass
import concourse.bacc as bacc
import concourse.tile as tile
from concourse import mybir
from concourse.bass_utils import run_bass_kernel_spmd
from bass_rust import add_dep_helper

# Problem shape (hardcoded; kernel.py must be self-contained)
N, C, T, V, M = 256, 3, 600, 25, 2
N_CORES = 8
NL = N // N_CORES       # 32 samples per core
NBLK = NL * C           # 96 (n_local, c) blocks per core
P = 120                 # partition rows per block (t-groups)
TS = T // P             # 5 t rows per partition row
FB = TS * V * M         # 250 block free width
W1 = 4 * V * M          # 200 = in-row motion pair columns
WVM = V * M             # 50

B_BLK = 12              # blocks per tile (= 4 samples)
N_TILES = NBLK // B_BLK  # 8
SPT = B_BLK // C        # 4 samples per tile -> 4 partition strips
BANK = 512              # fp32 per PSUM bank per partition
OUTW = FB + W1 + WVM    # 500 used columns per sample

F32 = mybir.dt.float32
BF16 = mybir.dt.bfloat16


def build_program():
    nc = bacc.Bacc("TRN2", target_bir_lowering=False, debug=False)
    # x and y interleaved per block so each tile is ONE dma_start (single
    # completion semaphore -> consumers stay within the HW wait-slot limit)
    # and the DMA source AP merges to 3 dims.
    xy_d = nc.dram_tensor("xy", [NBLK, 2, P, FB], F32, kind="ExternalInput").ap()
    out_d = nc.dram_tensor("out", [SPT, N_TILES, OUTW], F32, kind="ExternalOutput").ap()

    with tile.TileContext(nc) as tc, ExitStack() as ctx:
        io = ctx.enter_context(tc.tile_pool(name="io", bufs=4))
        shift = ctx.enter_context(tc.tile_pool(name="shift", bufs=4))
        work = ctx.enter_context(tc.tile_pool(name="work", bufs=2))
        singles = ctx.enter_context(tc.tile_pool(name="singles", bufs=1))
        psum_pool = ctx.enter_context(tc.tile_pool(name="psum", bufs=1, space="PSUM"))

        ones_t = singles.tile([P, 32], BF16)
        nc.vector.memset(ones_t, 1.0)

        # 1-element scratches for wait-absorbing joiner ops (see loop body)
        jscr_d = singles.tile([1, 1], F32, tag="jscr_d")
        jscr_x = singles.tile([1, 1], F32, tag="jscr_x")
        jscr_a = singles.tile([1, 1], F32, tag="jscr_a")
        jscr_b = singles.tile([1, 1], F32, tag="jscr_b")
        jscr_c = singles.tile([1, 1], F32, tag="jscr_c")

        psum_t = psum_pool.tile([128, N_TILES, BANK], F32)

        for i in range(N_TILES):
            b0 = i * B_BLK
            xy_t = io.tile([P, B_BLK, 2, FB], F32, tag="xy")
            nc.sync.dma_start(
                out=xy_t,
                in_=xy_d[b0 : b0 + B_BLK].rearrange("b s p f -> p (b s) f"),
            )
            x_t = xy_t[:, :, 0]
            y_t = xy_t[:, :, 1]

            # Hardware sync-wait slots are scarce (DMA: 1, compute: 2). Tiny
            # "joiner" ops absorb cross-proc waits first so the real ops'
            # waits are dominated: jd eats the xy DMA wait on DVE, jx eats
            # the xs DMA wait on DVE, ja1-3 eat the DVE waits on ACT.
            jd = nc.vector.tensor_copy(jscr_d, xy_t[0:1, 0, 0, 0:1])

            d_t = work.tile([P, B_BLK, FB], BF16, tag="d")
            i_d = nc.vector.tensor_sub(d_t, x_t, y_t)
            add_dep_helper(i_d.ins, jd.ins, sync=False, reason="jd first")

            d1_t = work.tile([P, B_BLK, W1], F32, tag="d1")
            i_d1 = nc.vector.tensor_sub(d1_t, x_t[:, :, WVM:FB], x_t[:, :, 0:W1])
            add_dep_helper(i_d1.ins, jd.ins, sync=False, reason="jd first")

            # strip copy keeps the xy tile's readers DVE-only (the load's WAR
            # must fit in the 1-wait DMA sync slot); the partition shift then
            # rides a small SBUF->SBUF DMA off the private strip tile.
            strip_t = work.tile([P, B_BLK, WVM], F32, tag="strip")
            i_st = nc.vector.tensor_copy(strip_t, x_t[:, :, 0:WVM])
            add_dep_helper(i_st.ins, jd.ins, sync=False, reason="jd first")
            xs_t = shift.tile([P - 1, B_BLK, WVM], F32, tag="xs")
            nc.sync.dma_start(out=xs_t, in_=strip_t[1:P])
            jx = nc.vector.tensor_copy(jscr_x, xs_t[0:1, 0, 0:1])
            d2_t = work.tile([P - 1, B_BLK, WVM], F32, tag="d2")
            i_d2 = nc.vector.tensor_sub(d2_t, xs_t, x_t[0 : P - 1, :, FB - WVM : FB])
            add_dep_helper(i_d2.ins, jx.ins, sync=False, reason="jx first")

            ja1 = nc.scalar.copy(jscr_a, d_t[0:1, 0, 0:1])
            ja2 = nc.scalar.copy(jscr_b, d1_t[0:1, 0, 0:1])
            ja3 = nc.scalar.copy(jscr_c, d2_t[0:1, 0, 0:1])
            sq_t = work.tile([P, B_BLK, FB], BF16, tag="sq")
            i_sq = nc.scalar.activation(sq_t, d_t, mybir.ActivationFunctionType.Square)
            a1_t = work.tile([P, B_BLK, W1], BF16, tag="a1")
            i_a1 = nc.scalar.activation(a1_t, d1_t, mybir.ActivationFunctionType.Abs)
            a2_t = work.tile([P - 1, B_BLK, WVM], BF16, tag="a2")
            i_a2 = nc.scalar.activation(a2_t, d2_t, mybir.ActivationFunctionType.Abs)
            for act_i in (i_sq, i_a1, i_a2):
                for jo in (ja1, ja2, ja3):
                    add_dep_helper(act_i.ins, jo.ins, sync=False, reason="ja first")

            for j in range(SPT):
                sp = 32 * j
                for c in range(C):
                    bb = C * j + c
                    nc.tensor.matmul(
                        psum_t[sp : sp + 32, i, 0:FB],
                        ones_t[:, 0:32],
                        sq_t[:, bb, :],
                        start=(c == 0),
                        stop=(c == C - 1),
                        tile_position=(0, sp),
                    )
                for c in range(C):
                    bb = C * j + c
                    nc.tensor.matmul(
                        psum_t[sp : sp + 32, i, FB : FB + W1],
                        ones_t[:, 0:32],
                        a1_t[:, bb, :],
                        start=(c == 0),
                        stop=(c == C - 1),
                        tile_position=(0, sp),
                    )
                for c in range(C):
                    bb = C * j + c
                    nc.tensor.matmul(
                        psum_t[sp : sp + 32, i, FB + W1 : OUTW],
                        ones_t[0 : P - 1, 0:32],
                        a2_t[:, bb, :],
                        start=(c == 0),
                        stop=(c == C - 1),
                        tile_position=(0, sp),
                    )

        collect = singles.tile([128, N_TILES, OUTW], F32)
        nc.scalar.copy(collect, psum_t[:, :, 0:OUTW])
        for j in range(SPT):
            nc.sync.dma_start(out=out_d[j : j + 1], in_=collect[32 * j : 32 * j + 1])

    nc.compile()
    return nc


def host_reduce(outs: np.ndarray) -> np.float32:
    """outs: (N_CORES, SPT, N_TILES, OUTW) f32 -> scalar loss."""
    o = outs.astype(np.float64)
    # sample n = 32*core + 4*i + j  <->  o[core, j, i]
    S = o[..., 0:FB].reshape(N_CORES, SPT, N_TILES, TS, V, M).sum(axis=(3, 5))
    A = o[..., FB : FB + W1].reshape(N_CORES, SPT, N_TILES, 4, V, M).sum(axis=(3, 5))
    A += o[..., FB + W1 : OUTW].reshape(N_CORES, SPT, N_TILES, V, M).sum(axis=4)
    num = (A * S).sum(axis=-1)
    den = A.sum(axis=-1)
    loss = (V * num / den).sum() / float(N * C * T * V * M)
    return np.float32(loss)


def make_in_maps(x, y):
    xr = x.reshape(N_CORES, NBLK, P, FB)
    yr = y.reshape(N_CORES, NBLK, P, FB)
    return [
        {"xy": np.stack([xr[k], yr[k]], axis=1)} for k in range(N_CORES)
    ]


_NC_CACHE = None


def kernel(x: np.ndarray, y: np.ndarray) -> np.ndarray:
    global _NC_CACHE
    x = np.asarray(x, dtype=np.float32)
    y = np.asarray(y, dtype=np.float32)
    in_maps = make_in_maps(x, y)
    if _NC_CACHE is None:
        _NC_CACHE = build_program()
    res = run_bass_kernel_spmd(_NC_CACHE, in_maps, list(range(N_CORES))).results
    outs = np.stack([res[k]["out"] for k in range(N_CORES)])
    return host_reduce(outs)
